# revision 1
# baseline (speedup 1.0000x reference)
"""CSWin attention Trainium2 kernel.

Shapes (hardcoded): B=8, H=W=64, N=4096, C=512, 8 heads (4 horizontal-stripe,
4 vertical-stripe), head_dim=64, stripe width SPLIT=8.

Sharding: data-parallel over batch B across the 8 NeuronCores (1 image/core).

On-chip strategy (per core, all matmuls bf16 with fp32 PSUM accumulation):
  - x [4096, 512] -> xT [512, 4096] (PE transposes, copies split ACT/DVE),
    channel-major.
  - qkvT [1536, 4096] = WqkvT @ xT (+bias folded into the PSUM->SBUF
    tensor_scalar_add copy).  v-half head channels are written in
    column-major token order so vertical stripes are contiguous too.
  - attention runs as two interleaved independent streams (h-half /
    v-half) of head-pairs, software-pipelined one pair ahead:
      * LePE depthwise conv: 9 K=128 diagonal matmuls with shifted
        2-level-AP rhs windows, accumulating v + conv(v) (+bias in the
        copy-out) in PSUM for a whole head pair at once;
      * scoresT = k-stationary matmul (head pairs packed onto disjoint
        PE row groups), exp on ScalarE straight out of PSUM (no
        max-subtraction needed: scores ~ N(0,1));
      * AV matmul with a ones-augmented transposed v_lepe so PSUM row 64
        is the softmax denominator for free;
      * normalization: DVE reciprocal of the denominator row, GPSIMD
        partition_broadcast + multiply scattering into concatT.
  - final proj: token-major PSUM matmuls from concatT + WprojT, bias via
    K=1 ones matmul, DMA out.
"""

import os
import numpy as np

import concourse.bass as bass
import concourse.bacc as bacc
import concourse.mybir as mybir
from concourse import bass_utils
from concourse.tile import TileContext
from concourse.masks import make_identity

F32 = mybir.dt.float32
BF16 = mybir.dt.bfloat16

B = 8
H = 64
W = 64
N = H * W          # 4096
C = 512
NH = 8             # heads
HD = 64            # head dim
SP = 8             # stripe width
NS = 8             # stripes per direction
SCALE = HD ** -0.5

_CACHE = {}


def _build_nc():
    nc = bacc.Bacc("TRN2", target_bir_lowering=False, debug=False)

    x_d = nc.dram_tensor("x", (N, C), F32, kind="ExternalInput").ap()
    wqkv_d = nc.dram_tensor("wqkv", (3 * C, C), F32, kind="ExternalInput").ap()
    bqkv_d = nc.dram_tensor("bqkv", (3 * C,), F32, kind="ExternalInput").ap()
    wproj_d = nc.dram_tensor("wproj", (C, C), F32, kind="ExternalInput").ap()
    bproj_d = nc.dram_tensor("bproj", (C,), F32, kind="ExternalInput").ap()
    lhw_d = nc.dram_tensor("lepe_h_w", (3, 3, 1, HD), F32, kind="ExternalInput").ap()
    lhb_d = nc.dram_tensor("lepe_h_b", (HD,), F32, kind="ExternalInput").ap()
    lvw_d = nc.dram_tensor("lepe_v_w", (3, 3, 1, HD), F32, kind="ExternalInput").ap()
    lvb_d = nc.dram_tensor("lepe_v_b", (HD,), F32, kind="ExternalInput").ap()
    y_d = nc.dram_tensor("y", (N, C), F32, kind="ExternalOutput").ap()

    with TileContext(nc) as tc:
        _emit(nc, tc, x_d, wqkv_d, bqkv_d, wproj_d, bproj_d,
              lhw_d, lhb_d, lvw_d, lvb_d, y_d)
    nc.compile()
    return nc


def _emit(nc, tc, x_d, wqkv_d, bqkv_d, wproj_d, bproj_d,
          lhw_d, lhb_d, lvw_d, lvb_d, y_d):
    import contextlib
    ctx = contextlib.ExitStack()
    with ctx:
        persist = ctx.enter_context(tc.tile_pool(name="persist", bufs=1))
        qkv_pool = ctx.enter_context(tc.tile_pool(name="qkvT", bufs=1))

        from concourse import library_config
        nc.gpsimd.load_library(library_config.proxy)

        # ---------------- constants / weights prep ----------------
        # identity [128, 128] bf16 for 128-row transposes
        id128 = persist.tile([128, 128], BF16, tag="id128")
        make_identity(nc, id128)
        # identity [64, 64] bf16 (rhs for the tiny [9, 64] weight transpose)
        id64 = persist.tile([64, 64], BF16, tag="id64")
        make_identity(nc, id64)


        # ---------------- xT, then qkvT [1536, 4096] ----------------
        # qkvT: 12 tiles [128, 4096]; tile jt holds channels
        # [128*jt, 128*jt+128): jt 0-3: q (heads 0-7), 4-7: k, 8-11: v.
        # Within each group tiles 0-1 = h-half heads (row-major tokens),
        # 2-3 = v-half heads (column-major token order t' = x*64 + y).
        qkvT = [qkv_pool.tile([128, N], BF16, name=f"qkvT{jt}", tag=f"qkvT{jt}") for jt in range(12)]
        with tc.tile_pool(name="xT", bufs=1) as xT_pool:
            xT = [xT_pool.tile([128, N], BF16, name=f"xT{cc}", tag=f"xT{cc}") for cc in range(4)]
            with tc.tile_pool(name="xload", bufs=2) as xload, \
                 tc.tile_pool(name="xt_psum", bufs=4, space="PSUM") as xt_psum:
                for tg in range(8):
                    rows_bf = []
                    for j in range(4):
                        tt = tg * 4 + j
                        xrow = xload.tile([128, C], F32, tag=f"xrow{j}")
                        nc.sync.dma_start(
                            out=xrow, in_=x_d[tt * 128:(tt + 1) * 128, :])
                        xrow_bf = xload.tile([128, C], BF16, tag=f"xrow_bf{j}")
                        nc.vector.tensor_copy(xrow_bf, xrow)
                        rows_bf.append(xrow_bf)
                    for cc in range(4):
                        ps = xt_psum.tile([128, 512], BF16, tag="xps")
                        for j in range(4):
                            nc.tensor.transpose(
                                ps[:, j * 128:(j + 1) * 128],
                                rows_bf[j][:, cc * 128:(cc + 1) * 128], id128)
                        if cc % 2 == 0:
                            nc.scalar.activation(
                                xT[cc][:, tg * 512:(tg + 1) * 512], ps,
                                mybir.ActivationFunctionType.Copy)
                        else:
                            nc.vector.tensor_copy(
                                xT[cc][:, tg * 512:(tg + 1) * 512], ps)

                # --- load + cast + transpose Wqkv -> WqkvT [c, j]: 4 tiles [128, 1536]
                # (casts on DVE; PSUM->SBUF copies on the otherwise-idle ScalarE)
                wqkvT = [persist.tile([128, 3 * C], BF16, name=f"wqkvT{cc}", tag=f"wqkvT{cc}")
                         for cc in range(4)]
                for jg in range(3):
                    rows_bf = []
                    for j in range(4):
                        jt = jg * 4 + j
                        wrow = xload.tile([128, C], F32, tag=f"xrow{j}")
                        nc.sync.dma_start(
                            out=wrow, in_=wqkv_d[jt * 128:(jt + 1) * 128, :])
                        wrow_bf = xload.tile([128, C], BF16, tag=f"xrow_bf{j}")
                        nc.vector.tensor_copy(wrow_bf, wrow)
                        rows_bf.append(wrow_bf)
                    for cc in range(4):
                        ps = xt_psum.tile([128, 512], BF16, tag="xps")
                        for j in range(4):
                            nc.tensor.transpose(
                                ps[:, j * 128:(j + 1) * 128],
                                rows_bf[j][:, cc * 128:(cc + 1) * 128], id128)
                        if cc % 2 == 0:
                            nc.scalar.activation(
                                wqkvT[cc][:, jg * 512:(jg + 1) * 512], ps,
                                mybir.ActivationFunctionType.Copy)
                        else:
                            nc.vector.tensor_copy(
                                wqkvT[cc][:, jg * 512:(jg + 1) * 512], ps)

                # --- Wproj -> WprojT [f, e]: 4 tiles [128, 512]
                wprojT = [persist.tile([128, C], BF16, name=f"wprojT{fc}", tag=f"wprojT{fc}")
                          for fc in range(4)]
                rows_bf = []
                for et in range(4):
                    wrow = xload.tile([128, C], F32, tag=f"xrow{et}")
                    nc.sync.dma_start(out=wrow, in_=wproj_d[et * 128:(et + 1) * 128, :])
                    wrow_bf = xload.tile([128, C], BF16, tag=f"xrow_bf{et}")
                    nc.vector.tensor_copy(wrow_bf, wrow)
                    rows_bf.append(wrow_bf)
                for fc in range(4):
                    ps = xt_psum.tile([128, 512], BF16, tag="xps")
                    for et in range(4):
                        nc.tensor.transpose(
                            ps[:, et * 128:(et + 1) * 128],
                            rows_bf[et][:, fc * 128:(fc + 1) * 128], id128)
                    nc.scalar.activation(
                        wprojT[fc], ps, mybir.ActivationFunctionType.Copy)

                # --- biases ---
                # bqkv per-partition: [128, 12] (partition p, col jt) = bqkv[jt*128+p]
                bqkv_sb = persist.tile([128, 12], F32, tag="bqkv")
                nc.sync.dma_start(out=bqkv_sb, in_=bqkv_d.rearrange("(a p) -> p a", p=128))
                # bproj as a bf16 row [1, 512] (K=1 matmul rhs)
                bproj_f32 = xload.tile([1, C], F32, tag="xrow0", name="bproj_f32")
                nc.sync.dma_start(out=bproj_f32, in_=bproj_d.rearrange("(a e) -> a e", a=1))
                bproj_sb = persist.tile([1, C], BF16, tag="bproj")
                nc.vector.tensor_copy(bproj_sb, bproj_f32)
                ones_row = persist.tile([1, 128], BF16, tag="ones_row")
                nc.vector.memset(ones_row, 1.0)
                # lepe biases [128, 1] (duplicated across both 64-row halves so one
                # op covers a head pair)
                lepe_b = []
                for name, d in (("lhb", lhb_d), ("lvb", lvb_d)):
                    t = persist.tile([128, 1], F32, name=name, tag=name)
                    nc.sync.dma_start(out=t[0:64, :], in_=d.rearrange("(p a) -> p a", a=1))
                    nc.sync.dma_start(out=t[64:128, :], in_=d.rearrange("(p a) -> p a", a=1))
                    lepe_b.append(t)

                # --- LePE diag weights ---
                # load [9, 64], transpose to wT [64, 9], duplicate to [128, 9],
                # then diag tiles [128, 64] (both 64-row halves hold the same diag).
                # diags[half][k] for taps k=0..8 ((dr,dc) row-major); center (k=4)
                # gets I added.
                diags = []
                for half, wsrc in ((0, lhw_d), (1, lvw_d)):
                    w9 = xload.tile([9, 64], F32, tag="w9")
                    nc.sync.dma_start(out=w9, in_=wsrc.rearrange("a b c d -> (a b c) d"))
                    w9_bf = xload.tile([9, 64], BF16, tag="w9bf")
                    nc.vector.tensor_copy(w9_bf, w9)
                    ps = xt_psum.tile([64, 9], BF16, tag="wTps", bufs=1)
                    nc.tensor.transpose(ps, w9_bf, id64[0:9, 0:9])
                    wT = persist.tile([128, 9], F32, tag=f"wT{half}")
                    nc.vector.tensor_copy(wT[0:64, :], ps)
                    nc.sync.dma_start(out=wT[64:128, :], in_=wT[0:64, :])
                    dh = []
                    nh = []
                    for k in range(9):
                        # tap index k -> (dr, dc); weight index depends on half:
                        # half 0 (horizontal): local (dr, dc) = (dy, dx) -> w[dy, dx]
                        # half 1 (vertical):   local (dr, dc) = (dx, dy) -> w[dy=dc, dx=dr]
                        dr, dc = k // 3 - 1, k % 3 - 1
                        if half == 0:
                            wi = (dr + 1) * 3 + (dc + 1)
                        else:
                            wi = (dc + 1) * 3 + (dr + 1)
                        # [128, 128] diagonal covering a head PAIR (weights repeat
                        # every 64 channels via the duplicated wT halves)
                        dt = persist.tile([128, 128], BF16, tag=f"diag{half}_{k}")
                        nc.vector.tensor_scalar_mul(dt, id128, wT[:, wi:wi + 1])
                        if k == 4:
                            nc.vector.tensor_add(dt, dt, id128)
                        dh.append(dt)
                    diags.append(dh)


            with tc.tile_pool(name="qkv_psum", bufs=4, space="PSUM") as qkv_psum:
                for jt in range(12):
                    vhalf = (jt % 4) >= 2
                    for nt in range(8):
                        ps = qkv_psum.tile([128, 512], F32, tag="qkvps")
                        for cc in range(4):
                            nc.tensor.matmul(
                                ps, wqkvT[cc][:, jt * 128:(jt + 1) * 128],
                                xT[cc][:, nt * 512:(nt + 1) * 512],
                                start=(cc == 0), stop=(cc == 3))
                        if vhalf:
                            # scatter token chunk (rows y in [8nt, 8nt+8),
                            # all x) into column-major: addr = x*64 + y
                            out_ap = bass.AP(
                                tensor=qkvT[jt].tensor,
                                offset=qkvT[jt].offset + 8 * nt,
                                ap=[qkvT[jt].ap[0], [1, 8], [64, 64]])
                        else:
                            out_ap = qkvT[jt][:, nt * 512:(nt + 1) * 512]
                        nc.vector.tensor_scalar_add(
                            out_ap, ps, bqkv_sb[:, jt:jt + 1])

        # ---------------- attention ----------------
        concatT = [persist.tile([128, N], BF16, name=f"concatT{fc}", tag=f"concatT{fc}")
                   for fc in range(4)]

        # pair list: 32 head-pairs; each pair = 2 heads sharing a qkvT tile
        pairs = [(half, s, hp)
                 for half in range(2) for s in range(NS) for hp in range(2)]

        # PSUM budget (8 banks): scores 3x[128,1024] (6) + LePE/transpose
        # shared slot (1) + AV output (1)
        with tc.tile_pool(name="sc_psum", bufs=3, space="PSUM") as sc_psum, \
             tc.tile_pool(name="lp_psum", bufs=1, space="PSUM") as lp_psum, \
             tc.tile_pool(name="oa_psum", bufs=1, space="PSUM") as oa_psum, \
             tc.tile_pool(name="att", bufs=4) as att, \
             tc.tile_pool(name="lepe_sb", bufs=4) as lepe_sb, \
             tc.tile_pool(name="norm_sb", bufs=5) as norm_sb:

            def emit_lepe(pi):
                """LePE for pair pi -> vlsb SBUF pair slab [128, 512].

                9 K=128 diagonal matmuls accumulating in PSUM; the center
                tap goes first (full range, start=True) so every element
                has has_written set; shifted taps write only their valid
                (rows x cols) window via 2-level APs."""
                half, s, hp = pairs[pi]
                tok0 = s * 512
                vt = qkvT[8 + half * 2 + hp]
                vsp = vt[:, tok0:tok0 + 512]
                vl = lp_psum.tile([128, 512], F32, tag="lps", name="vl")
                order = [4, 0, 1, 2, 3, 5, 6, 7, 8]
                for ki, k in enumerate(order):
                    dr, dc = k // 3 - 1, k % 3 - 1
                    r0 = max(0, -dr)
                    nr = 8 - abs(dr)
                    x0 = max(0, -dc)
                    nx = 64 - abs(dc)
                    o_off = r0 * 64 + x0
                    i_off = (r0 + dr) * 64 + (x0 + dc)
                    if dc == 0:
                        out_ap = vl[:, o_off:o_off + nr * 64]
                        in_ap = vsp[:, i_off:i_off + nr * 64]
                    else:
                        out_ap = bass.AP(
                            tensor=vl.tensor, offset=vl.offset + o_off,
                            ap=[vl.ap[0], [64, nr], [1, nx]])
                        in_ap = bass.AP(
                            tensor=vsp.tensor, offset=vsp.offset + i_off,
                            ap=[vsp.ap[0], [64, nr], [1, nx]])
                    nc.tensor.matmul(
                        out_ap, diags[half][k], in_ap,
                        start=(ki == 0), stop=(ki == len(order) - 1),
                        skip_group_check=True)
                # copy + bias -> SBUF bf16 (pair slab)
                vlsb = lepe_sb.tile([128, 512], BF16, tag="vlsb", name="vlsb")
                nc.vector.tensor_scalar_add(vlsb, vl, lepe_b[half])
                return vlsb

            def emit_vna(pi, vlsb):
                """Transpose pair slab + build pair v_nat_aug [128, 520]:
                chunk (jc, head) at cols (jc*2+head)*65, col 64 = ones."""
                vnp = lp_psum.tile([128, 512], BF16, tag="lps", name="vnp")
                for jc in range(4):
                    nc.tensor.transpose(
                        vnp[:, jc * 128:(jc + 1) * 128],
                        vlsb[:, jc * 128:(jc + 1) * 128], id128)
                vna = lepe_sb.tile([128, 520], BF16, tag="vna", name="vna")
                # one copy for both heads: in (jc, head, d) -> out chunks
                vna_data = bass.AP(
                    tensor=vna.tensor, offset=vna.offset,
                    ap=[vna.ap[0], [130, 4], [65, 2], [1, 64]])
                nc.vector.tensor_copy(vna_data, vnp)
                vna_ones = bass.AP(
                    tensor=vna.tensor, offset=vna.offset + 64,
                    ap=[vna.ap[0], [130, 4], [65, 2]])
                nc.vector.memset(vna_ones, 1.0)
                return vna

            def emit_pair_scores(pi):
                """scoresT + exp for BOTH heads of pair pi, emitted
                chunk-interleaved: h0 uses PE rows 0-63 and h1 rows 64-127,
                so adjacent matmuls run concurrently on disjoint row
                groups.  Returns (esb_h0, esb_h1)."""
                half, s, hp = pairs[pi]
                tok0 = s * 512
                jt_off = half * 2 + hp
                esbs = []
                qkss = []
                for hh in range(2):
                    pbase = hh * 64
                    qkss.append((
                        qkvT[jt_off][pbase:pbase + 64, tok0:tok0 + 512],
                        qkvT[4 + jt_off][pbase:pbase + 64, tok0:tok0 + 512]))
                    esbs.append(att.tile([128, 2048], BF16, tag="esb",
                                         name="esb"))
                for sh in range(2):
                    for hh in range(2):
                        qs, ks = qkss[hh]
                        sps = sc_psum.tile([128, 1024], F32, tag="sps",
                                           name="sps")
                        for jj in range(2):
                            jc = 2 * sh + jj
                            nc.tensor.matmul(
                                sps[:, jj * 512:(jj + 1) * 512],
                                ks[:, jc * 128:(jc + 1) * 128], qs,
                                start=True, stop=True)
                        # exp; no max subtraction needed (scores ~ N(0,1))
                        nc.scalar.activation(
                            esbs[hh][:, sh * 1024:(sh + 1) * 1024], sps,
                            mybir.ActivationFunctionType.Exp,
                            bias=0.0, scale=SCALE)
                return esbs

            def emit_av_norm(pi, hh, vna, esb):
                half, s, hp = pairs[pi]
                tok0 = s * 512
                jt_off = half * 2 + hp
                pbase = hh * 64
                # AV: outT_aug rows 0-63 = result, row 64 = denominator
                oa = oa_psum.tile([65, 512], F32, tag="oa", name="oa")
                for jc in range(4):
                    nc.tensor.matmul(
                        oa, vna[:, (jc * 2 + hh) * 65:(jc * 2 + hh) * 65 + 65],
                        esb[:, jc * 512:(jc + 1) * 512],
                        start=(jc == 0), stop=(jc == 3))
                # normalization: concatT <- out * (1/denom)
                osb = norm_sb.tile([64, 512], BF16, tag="osb", name="osb")
                nc.vector.tensor_copy(osb, oa[0:64, :])
                rec = norm_sb.tile([1, 512], F32, tag="rec", name="rec")
                nc.vector.reciprocal(rec, oa[64:65, :])
                denb = norm_sb.tile([64, 512], F32, tag="denb", name="denb")
                nc.gpsimd.partition_broadcast(denb, rec)
                cfc = concatT[jt_off]
                if half == 0:
                    out_ap = cfc[pbase:pbase + 64, tok0:tok0 + 512]
                else:
                    # scatter col-major stripe to row-major:
                    # local j = x*64 + y -> t = y*64 + 8s + x
                    out_ap = bass.AP(
                        tensor=cfc.tensor,
                        offset=cfc.offset + pbase * cfc.ap[0][0] + 8 * s,
                        ap=[[cfc.ap[0][0], 64], [1, 8], [64, 64]])
                nc.gpsimd.tensor_tensor(
                    out_ap, osb, denb, mybir.AluOpType.mult)

            # two independent streams (h-half pairs 0-15, v-half pairs
            # 16-31) interleaved so one stream's compute fills the other's
            # semaphore-latency bubbles; within each stream, LePE/
            # transposes for pair i+1 are software-pipelined between the
            # heads of pair i.
            streams = [list(range(0, 16)), list(range(16, 32))]
            vna_cur = []
            for st in (0, 1):
                vlsb0 = emit_lepe(streams[st][0])
                vna_cur.append(emit_vna(streams[st][0], vlsb0))
            nsteps = len(streams[0])
            for i in range(nsteps):
                p = [streams[0][i], streams[1][i]]
                nxt = [streams[st][i + 1] if i + 1 < nsteps else None
                       for st in (0, 1)]
                esbA = emit_pair_scores(p[0])
                vlsb_next = [None, None]
                if nxt[0] is not None:
                    vlsb_next[0] = emit_lepe(nxt[0])
                emit_av_norm(p[0], 0, vna_cur[0], esbA[0])
                esbB = emit_pair_scores(p[1])
                emit_av_norm(p[0], 1, vna_cur[0], esbA[1])
                if nxt[1] is not None:
                    vlsb_next[1] = emit_lepe(nxt[1])
                emit_av_norm(p[1], 0, vna_cur[1], esbB[0])
                if nxt[0] is not None:
                    vna_cur[0] = emit_vna(nxt[0], vlsb_next[0])
                emit_av_norm(p[1], 1, vna_cur[1], esbB[1])
                if nxt[1] is not None:
                    vna_cur[1] = emit_vna(nxt[1], vlsb_next[1])

        # ---------------- proj ----------------
        with tc.tile_pool(name="pj_psum", bufs=4, space="PSUM") as pj_psum, \
             tc.tile_pool(name="pj", bufs=5) as pj:
            for tt in range(32):
                ps = pj_psum.tile([128, C], F32, tag="pjps")
                for fc in range(4):
                    nc.tensor.matmul(
                        ps, concatT[fc][:, tt * 128:(tt + 1) * 128],
                        wprojT[fc],
                        start=(fc == 0), stop=False,
                        skip_group_check=True)
                # bias via K=1 ones matmul
                nc.tensor.matmul(
                    ps, ones_row, bproj_sb,
                    start=False, stop=True, skip_group_check=True)
                osb = pj.tile([128, C], F32, tag="pjout")
                if tt % 2 == 0:
                    nc.vector.tensor_copy(osb, ps)
                else:
                    nc.scalar.activation(
                        osb, ps, mybir.ActivationFunctionType.Copy)
                nc.sync.dma_start(
                    out=y_d[tt * 128:(tt + 1) * 128, :], in_=osb)


def _get_nc():
    if "nc" not in _CACHE:
        _CACHE["nc"] = _build_nc()
    return _CACHE["nc"]


def kernel(**inputs):
    x = np.asarray(inputs["x"], dtype=np.float32)
    names = {
        "wqkv": "Wqkv", "bqkv": "bqkv", "wproj": "Wproj", "bproj": "bproj",
        "lepe_h_w": "lepe_h_w", "lepe_h_b": "lepe_h_b",
        "lepe_v_w": "lepe_v_w", "lepe_v_b": "lepe_v_b",
    }
    shared = {k: np.ascontiguousarray(np.asarray(inputs[v], dtype=np.float32))
              for k, v in names.items()}
    nc = _get_nc()
    in_maps = []
    for b in range(B):
        m = dict(shared)
        m["x"] = np.ascontiguousarray(x[b])
        in_maps.append(m)
    res = bass_utils.run_bass_kernel_spmd(nc, in_maps, core_ids=list(range(B)))
    out = np.stack([res.results[b]["y"] for b in range(B)], axis=0)
    return out.astype(np.float32)


if __name__ == "__main__":
    rng = np.random.default_rng(0)
    ins = {
        "x": rng.standard_normal((B, N, C), dtype=np.float32),
        "Wqkv": rng.standard_normal((3 * C, C), dtype=np.float32) * C ** -0.5,
        "bqkv": np.zeros(3 * C, np.float32),
        "Wproj": rng.standard_normal((C, C), dtype=np.float32) * C ** -0.5,
        "bproj": np.zeros(C, np.float32),
        "lepe_h_w": rng.standard_normal((3, 3, 1, HD), dtype=np.float32) / 3,
        "lepe_h_b": np.zeros(HD, np.float32),
        "lepe_v_w": rng.standard_normal((3, 3, 1, HD), dtype=np.float32) / 3,
        "lepe_v_b": np.zeros(HD, np.float32),
        "H": np.int64(H), "W": np.int64(W),
    }
    out = kernel(**ins)
    print(out.shape, out.dtype)



# revision 8
# speedup vs baseline: 1.0763x; 1.0763x over previous
"""CSWin attention Trainium2 kernel (v2 — flipped AV + off-PE LePE).

Shapes (hardcoded): B=8, H=W=64, N=4096, C=512, 8 heads (4 horizontal-stripe,
4 vertical-stripe), head_dim=64, stripe width SPLIT=8.

Sharding: data-parallel over batch B across the 8 NeuronCores (1 image/core).

Key ideas vs the v1 kernel (all matmuls bf16 with fp32 PSUM):
  - qkv prologue: W loads/transposes first, then an 8-step pipeline per
    512-token chunk: batched x DMA -> DVE cast -> PE transpose -> qkv
    matmuls -> PSUM->SBUF bias copies split between ScalarE and DVE.
  - LePE depthwise conv is mostly OFF the tensor engine: a DVE
    scalar_tensor_tensor in-place chain (center + 4 wide taps) plus the 4
    corner taps as K=128 diagonal matmuls on PE (their windows are
    smallest); one DVE op combines PSUM + chain + bias into bf16 vlsb.
  - v_lepe transposed to token-major via XBAR DMA transpose (14ns/tile on
    the otherwise idle DMA engines) instead of PE transposes + DVE copies.
  - AV is FLIPPED: out[tq, d] = esb^T @ v_aug uses all 128 PE output
    partitions (65 cols/matmul instead of 512: 2x fewer PE cycles), and the
    softmax denominator lands in a per-PARTITION column via interleaved
    1-col ones-matmuls (PSUM zero-region rule: single start=True on the
    first matmul per bank).  Normalization = batched DVE reciprocal [128,4]
    + one 0-stride-broadcast tensor_tensor per head-stripe.
  - normalized outputs collect token-major in TTc tiles; big XBAR DMA
    transposes produce channel-major concatT (v-half goes through a small
    col-major staging tile + DVE scatter-copy for the global token order).
  - proj bias via tensor_tensor add against a pre-broadcast bias tile
    (Pool/DVE), not a K=1 PE matmul.
"""

import numpy as np

import concourse.bass as bass
import concourse.bacc as bacc
import concourse.mybir as mybir
from concourse import bass_utils
from concourse.tile import TileContext
from concourse.masks import make_identity

F32 = mybir.dt.float32
BF16 = mybir.dt.bfloat16
ALU = mybir.AluOpType

B = 8
H = 64
W = 64
N = H * W          # 4096
C = 512
NH = 8             # heads
HD = 64            # head dim
SP = 8             # stripe width
NS = 8             # stripes per direction
SCALE = HD ** -0.5

# taps on PE as diagonal matmuls: center first (full width => owns the PSUM
# zero region) + 3 corner taps (smallest windows); the rest run in the DVE
# scalar_tensor_tensor chain (first DVE tap k=1 initializes acc + memset).
PE_TAPS = (4, 0, 2, 6)
DVE_TAPS = (1, 3, 5, 7, 8)

_CACHE = {}


def _win(t, off, nr, nx):
    return bass.AP(tensor=t.tensor, offset=t.offset + off,
                   ap=[t.ap[0], [64, nr], [1, nx]])


def _build_nc():
    nc = bacc.Bacc("TRN2", target_bir_lowering=False, debug=False)

    x_d = nc.dram_tensor("x", (N, C), F32, kind="ExternalInput").ap()
    wqkv_d = nc.dram_tensor("wqkv", (3 * C, C), F32, kind="ExternalInput").ap()
    bqkv_d = nc.dram_tensor("bqkv", (3 * C,), F32, kind="ExternalInput").ap()
    wproj_d = nc.dram_tensor("wproj", (C, C), F32, kind="ExternalInput").ap()
    bproj_d = nc.dram_tensor("bproj", (C,), F32, kind="ExternalInput").ap()
    lhw_d = nc.dram_tensor("lepe_h_w", (3, 3, 1, HD), F32, kind="ExternalInput").ap()
    lhb_d = nc.dram_tensor("lepe_h_b", (HD,), F32, kind="ExternalInput").ap()
    lvw_d = nc.dram_tensor("lepe_v_w", (3, 3, 1, HD), F32, kind="ExternalInput").ap()
    lvb_d = nc.dram_tensor("lepe_v_b", (HD,), F32, kind="ExternalInput").ap()
    y_d = nc.dram_tensor("y", (N, C), F32, kind="ExternalOutput").ap()

    with TileContext(nc) as tc:
        _emit(nc, tc, x_d, wqkv_d, bqkv_d, wproj_d, bproj_d,
              lhw_d, lhb_d, lvw_d, lvb_d, y_d)
    nc.compile()
    return nc


def _emit(nc, tc, x_d, wqkv_d, bqkv_d, wproj_d, bproj_d,
          lhw_d, lhb_d, lvw_d, lvb_d, y_d):
    import contextlib
    ctx = contextlib.ExitStack()
    with ctx:
        persist = ctx.enter_context(tc.tile_pool(name="persist", bufs=1))
        qkv_pool = ctx.enter_context(tc.tile_pool(name="qkvT", bufs=1))
        ttc_pool = ctx.enter_context(tc.tile_pool(name="ttc", bufs=1))

        from concourse import library_config
        nc.gpsimd.load_library(library_config.proxy)

        # ---------------- constants / weights prep ----------------
        id128 = persist.tile([128, 128], BF16, tag="id128")
        make_identity(nc, id128)
        id64 = persist.tile([64, 64], BF16, tag="id64")
        make_identity(nc, id64)
        ones_col = persist.tile([128, 1], BF16, tag="ones_col")
        nc.vector.memset(ones_col, 1.0)

        qkvT = [qkv_pool.tile([128, N], BF16, name=f"qkvT{jt}", tag=f"qkvT{jt}")
                for jt in range(12)]
        # token-major collect tiles (one per attention output pair/fc chunk):
        # [tsub, b*128 + d_pair]; h-half cols in global token order, v-half in
        # col-major stripe order.
        TTc = [ttc_pool.tile([128, N], BF16, name=f"TTc{fc}", tag=f"TTc{fc}")
               for fc in range(4)]

        with tc.tile_pool(name="xT", bufs=1) as xT_pool:
            xT = [xT_pool.tile([128, N], BF16, name=f"xT{cc}", tag=f"xT{cc}")
                  for cc in range(4)]
            with tc.tile_pool(name="xload", bufs=2) as xload, \
                 tc.tile_pool(name="xcast", bufs=2) as xcast, \
                 tc.tile_pool(name="xt_psum", bufs=3, space="PSUM") as xt_psum, \
                 tc.tile_pool(name="qkv_psum", bufs=4, space="PSUM") as qkv_psum:

                # --- Wqkv: 3 batched DMAs + casts + PE transposes ---
                wqkvT = [persist.tile([128, 3 * C], BF16, name=f"wqkvT{cc}",
                                      tag=f"wqkvT{cc}") for cc in range(4)]
                for jg in range(3):
                    wrow = xload.tile([128, 4, C], F32, tag="xrow")
                    nc.sync.dma_start(
                        out=wrow,
                        in_=wqkv_d.rearrange("(a p) c -> p a c", p=128)[
                            :, jg * 4:(jg + 1) * 4, :])
                    wrow_bf = xcast.tile([128, 4, C], BF16, tag="xrow_bf")
                    for j in range(4):
                        nc.vector.tensor_copy(wrow_bf[:, j], wrow[:, j])
                    for cc in range(4):
                        ps = xt_psum.tile([128, 512], BF16, tag="xps")
                        for j in range(4):
                            nc.tensor.transpose(
                                ps[:, j * 128:(j + 1) * 128],
                                wrow_bf[:, j, cc * 128:(cc + 1) * 128], id128)
                        if cc % 2 == 0:
                            nc.scalar.activation(
                                wqkvT[cc][:, jg * 512:(jg + 1) * 512], ps,
                                mybir.ActivationFunctionType.Copy)
                        else:
                            nc.vector.tensor_copy(
                                wqkvT[cc][:, jg * 512:(jg + 1) * 512], ps)

                # --- Wproj ---
                wprojT = [persist.tile([128, C], BF16, name=f"wprojT{fc}",
                                       tag=f"wprojT{fc}") for fc in range(4)]
                wrow = xload.tile([128, 4, C], F32, tag="xrow")
                nc.sync.dma_start(
                    out=wrow, in_=wproj_d.rearrange("(a p) c -> p a c", p=128))
                wrow_bf = xcast.tile([128, 4, C], BF16, tag="xrow_bf")
                for j in range(4):
                    nc.vector.tensor_copy(wrow_bf[:, j], wrow[:, j])
                for fc in range(4):
                    ps = xt_psum.tile([128, 512], BF16, tag="xps")
                    for et in range(4):
                        nc.tensor.transpose(
                            ps[:, et * 128:(et + 1) * 128],
                            wrow_bf[:, et, fc * 128:(fc + 1) * 128], id128)
                    nc.scalar.activation(
                        wprojT[fc], ps, mybir.ActivationFunctionType.Copy)

                # --- biases ---
                bqkv_sb = persist.tile([128, 12], F32, tag="bqkv")
                nc.sync.dma_start(out=bqkv_sb,
                                  in_=bqkv_d.rearrange("(a p) -> p a", p=128))
                # bproj as a bf16 row (K=1 matmul rhs) + ones row
                bproj_row = persist.tile([1, C], F32, tag="bproj_row")
                nc.sync.dma_start(out=bproj_row,
                                  in_=bproj_d.rearrange("(a e) -> a e", a=1))
                bproj_sb = persist.tile([1, C], BF16, tag="bproj_sb")
                nc.vector.tensor_copy(bproj_sb, bproj_row)
                ones_row = persist.tile([1, 128], BF16, tag="ones_row")
                nc.vector.memset(ones_row, 1.0)
                # lepe biases [128, 1] (dup across both 64-row halves)
                lepe_b = []
                for name, d in (("lhb", lhb_d), ("lvb", lvb_d)):
                    t = persist.tile([128, 1], F32, name=name, tag=name)
                    nc.sync.dma_start(out=t[0:64, :],
                                      in_=d.rearrange("(p a) -> p a", a=1))
                    nc.sync.dma_start(out=t[64:128, :],
                                      in_=d.rearrange("(p a) -> p a", a=1))
                    lepe_b.append(t)

                # --- LePE weights: wT [128, 9] f32 per half (center col +1);
                #     diag tiles for the PE taps.
                lepw = []
                diags = []
                for half, wsrc in ((0, lhw_d), (1, lvw_d)):
                    w9 = xload.tile([9, 64], F32, tag="w9")
                    nc.sync.dma_start(out=w9,
                                      in_=wsrc.rearrange("a b c d -> (a b c) d"))
                    w9_bf = xcast.tile([9, 64], BF16, tag="w9bf")
                    nc.vector.tensor_copy(w9_bf, w9)
                    ps = xt_psum.tile([64, 9], BF16, tag="wTps", bufs=1)
                    nc.tensor.transpose(ps, w9_bf, id64[0:9, 0:9])
                    wT = persist.tile([128, 9], F32, tag=f"wT{half}")
                    nc.vector.tensor_copy(wT[0:64, :], ps)
                    nc.sync.dma_start(out=wT[64:128, :], in_=wT[0:64, :])
                    # wi mapping per tap k (see baseline): half 0 direct,
                    # half 1 swapped.
                    wis = []
                    for k in range(9):
                        dr, dc = k // 3 - 1, k % 3 - 1
                        wis.append((dr + 1) * 3 + (dc + 1) if half == 0
                                   else (dc + 1) * 3 + (dr + 1))
                    # fold identity into the center weight
                    wic = wis[4]
                    nc.vector.tensor_scalar_add(wT[:, wic:wic + 1],
                                                wT[:, wic:wic + 1], 1.0)
                    lepw.append((wT, wis))
                    dh = {}
                    for k in PE_TAPS:
                        dt = persist.tile([128, 128], BF16, tag=f"diag{half}_{k}")
                        nc.vector.tensor_scalar_mul(dt, id128,
                                                    wT[:, wis[k]:wis[k] + 1])
                        dh[k] = dt
                    diags.append(dh)

                # --- x: per-chunk pipeline: load -> cast -> transpose -> qkv ---
                for tg in range(8):
                    xrow = xload.tile([128, 4, C], F32, tag="xrow")
                    nc.sync.dma_start(
                        out=xrow,
                        in_=x_d.rearrange("(a p) c -> p a c", p=128)[
                            :, tg * 4:(tg + 1) * 4, :])
                    xrow_bf = xcast.tile([128, 4, C], BF16, tag="xrow_bf")
                    for j in range(4):
                        nc.vector.tensor_copy(xrow_bf[:, j], xrow[:, j])
                    for cc in range(4):
                        ps = xt_psum.tile([128, 512], BF16, tag="xps")
                        for j in range(4):
                            nc.tensor.transpose(
                                ps[:, j * 128:(j + 1) * 128],
                                xrow_bf[:, j, cc * 128:(cc + 1) * 128], id128)
                        if cc % 2 == 0:
                            nc.scalar.activation(
                                xT[cc][:, tg * 512:(tg + 1) * 512], ps,
                                mybir.ActivationFunctionType.Copy)
                        else:
                            nc.vector.tensor_copy(
                                xT[cc][:, tg * 512:(tg + 1) * 512], ps)
                    # qkv matmuls for this token chunk
                    for jt in range(12):
                        vhalf = (jt % 4) >= 2
                        ps = qkv_psum.tile([128, 512], F32, tag="qkvps")
                        for cc in range(4):
                            nc.tensor.matmul(
                                ps, wqkvT[cc][:, jt * 128:(jt + 1) * 128],
                                xT[cc][:, tg * 512:(tg + 1) * 512],
                                start=(cc == 0), stop=(cc == 3))
                        if vhalf:
                            out_ap = bass.AP(
                                tensor=qkvT[jt].tensor,
                                offset=qkvT[jt].offset + 8 * tg,
                                ap=[qkvT[jt].ap[0], [1, 8], [64, 64]])
                            nc.vector.tensor_scalar_add(
                                out_ap, ps, bqkv_sb[:, jt:jt + 1])
                        else:
                            out_ap = qkvT[jt][:, tg * 512:(tg + 1) * 512]
                            if jt % 2 == 0:
                                nc.scalar.activation(
                                    out_ap, ps,
                                    mybir.ActivationFunctionType.Identity,
                                    bias=bqkv_sb[:, jt:jt + 1])
                            else:
                                nc.vector.tensor_scalar_add(
                                    out_ap, ps, bqkv_sb[:, jt:jt + 1])

        # ---------------- attention ----------------
        pairs = [(half, s, hp)
                 for half in range(2) for s in range(NS) for hp in range(2)]

        with tc.tile_pool(name="sc_psum", bufs=2, space="PSUM") as sc_psum, \
             tc.tile_pool(name="lp_psum", bufs=2, space="PSUM") as lp_psum, \
             tc.tile_pool(name="oa_psum", bufs=2, space="PSUM") as oa_psum, \
             tc.tile_pool(name="att", bufs=4) as att, \
             tc.tile_pool(name="lepe_sb", bufs=3) as lepe_sb, \
             tc.tile_pool(name="norm_sb", bufs=4) as norm_sb:

            def emit_lepe(pi):
                """LePE for pair pi: PE corner taps into PSUM + DVE chain;
                returns (vl_psum, acc) to be combined by emit_vlsb."""
                half, s, hp = pairs[pi]
                tok0 = s * 512
                vt = qkvT[8 + half * 2 + hp]
                vsp = vt[:, tok0:tok0 + 512]
                wT, wis = lepw[half]

                def geom(k):
                    dr, dc = k // 3 - 1, k % 3 - 1
                    r0 = max(0, -dr)
                    nr = 8 - abs(dr)
                    x0 = max(0, -dc)
                    nx = 64 - abs(dc)
                    return r0 * 64 + x0, (r0 + dr) * 64 + (x0 + dc), nr, nx

                # PE: center (full width, start) + corner taps in PSUM
                vl = lp_psum.tile([128, 512], F32, tag="lps", name="vl")
                nc.tensor.matmul(vl, diags[half][4], vsp,
                                 start=True, stop=False, skip_group_check=True)
                for ki, k in enumerate(PE_TAPS[1:]):
                    o_off, i_off, nr, nx = geom(k)
                    nc.tensor.matmul(
                        _win(vl, o_off, nr, nx), diags[half][k],
                        _win(vsp, i_off, nr, nx),
                        start=False, stop=(ki == len(PE_TAPS) - 2),
                        skip_group_check=True)

                # DVE chain (taps 1,3,5,7,8); first tap initializes acc
                acc = lepe_sb.tile([128, 512], F32, tag="acc", name="acc")
                o_off, i_off, nr, nx = geom(DVE_TAPS[0])
                nc.vector.tensor_scalar_mul(
                    _win(acc, o_off, nr, nx), _win(vsp, i_off, nr, nx),
                    wT[:, wis[DVE_TAPS[0]]:wis[DVE_TAPS[0]] + 1])
                # zero the complement (tap k=1 covers rows 1..7 full cols)
                nc.vector.memset(acc[:, 0:64], 0.0)
                for k in DVE_TAPS[1:]:
                    o_off, i_off, nr, nx = geom(k)
                    nc.vector.scalar_tensor_tensor(
                        _win(acc, o_off, nr, nx), _win(vsp, i_off, nr, nx),
                        wT[:, wis[k]:wis[k] + 1], _win(acc, o_off, nr, nx),
                        ALU.mult, ALU.add)
                return vl, acc

            def emit_vlsb(pi, vl, acc):
                """combine PSUM corner taps + DVE chain + bias -> bf16."""
                half, s, hp = pairs[pi]
                vlsb = lepe_sb.tile([128, 512], BF16, tag="vlsb", name="vlsb")
                nc.vector.scalar_tensor_tensor(
                    vlsb, vl, lepe_b[half], acc, ALU.add, ALU.add)
                return vlsb

            def emit_vna(pi, vlsb):
                """token-major v_lepe via XBAR DMA transpose: [128,4,128]."""
                vna = lepe_sb.tile([128, 4, 128], BF16, tag="vna", name="vna")
                nc.sync.dma_start_transpose(vna, vlsb)
                return vna

            def emit_pair_scores(pi):
                half, s, hp = pairs[pi]
                tok0 = s * 512
                jt_off = half * 2 + hp
                esbs = []
                qkss = []
                for hh in range(2):
                    pbase = hh * 64
                    qkss.append((
                        qkvT[jt_off][pbase:pbase + 64, tok0:tok0 + 512],
                        qkvT[4 + jt_off][pbase:pbase + 64, tok0:tok0 + 512]))
                    esbs.append(att.tile([128, 2048], BF16, tag="esb",
                                         name="esb"))
                for sh in range(2):
                    for hh in range(2):
                        qs, ks = qkss[hh]
                        sps = sc_psum.tile([128, 1024], F32, tag="sps",
                                           name="sps")
                        for jj in range(2):
                            jc = 2 * sh + jj
                            nc.tensor.matmul(
                                sps[:, jj * 512:(jj + 1) * 512],
                                ks[:, jc * 128:(jc + 1) * 128], qs,
                                start=True, stop=True)
                        nc.scalar.activation(
                            esbs[hh][:, sh * 1024:(sh + 1) * 1024], sps,
                            mybir.ActivationFunctionType.Exp,
                            bias=0.0, scale=SCALE)
                return esbs

            def emit_av_norm(pi, hh, vna, esb):
                half, s, hp = pairs[pi]
                fc = half * 2 + hp
                # flipped AV: oa[tq-sub, u*65 + (0:64 | 64)] over 4 u chunks
                oa = oa_psum.tile([128, 260], F32, tag="oa", name="oa")
                first = True
                for u in range(4):
                    for jc in range(4):
                        lhsT = esb[:, jc * 512 + u * 128:
                                   jc * 512 + u * 128 + 128]
                        nc.tensor.matmul(
                            oa[:, u * 65:u * 65 + 64], lhsT,
                            vna[:, jc, hh * 64:hh * 64 + 64],
                            start=first, stop=False, skip_group_check=True)
                        first = False
                        nc.tensor.matmul(
                            oa[:, u * 65 + 64:u * 65 + 65], lhsT, ones_col,
                            start=False, stop=(u == 3 and jc == 3),
                            skip_group_check=True)
                # batched normalization
                rr = norm_sb.tile([128, 4], F32, tag="rr", name="rr")
                nc.vector.reciprocal(
                    rr, bass.AP(tensor=oa.tensor, offset=oa.offset + 64,
                                ap=[oa.ap[0], [65, 4]]))
                oa_data = bass.AP(tensor=oa.tensor, offset=oa.offset,
                                  ap=[oa.ap[0], [65, 4], [1, 64]])
                rr_b = bass.AP(tensor=rr.tensor, offset=rr.offset,
                               ap=[rr.ap[0], [1, 4], [0, 64]])
                # out: TTc[fc] cols (4s+u)*128 + 64*hh + [0:64)
                out_ap = bass.AP(
                    tensor=TTc[fc].tensor,
                    offset=TTc[fc].offset + (4 * s) * 128 + 64 * hh,
                    ap=[TTc[fc].ap[0], [128, 4], [1, 64]])
                nc.vector.tensor_tensor(out_ap, oa_data, rr_b, ALU.mult)

            # two interleaved streams, LePE/vna pipelined one pair ahead
            streams = [list(range(0, 16)), list(range(16, 32))]
            vna_cur = [None, None]
            for st in (0, 1):
                vl0, acc0 = emit_lepe(streams[st][0])
                vna_cur[st] = emit_vna(streams[st][0],
                                       emit_vlsb(streams[st][0], vl0, acc0))
            nsteps = len(streams[0])
            for i in range(nsteps):
                p = [streams[0][i], streams[1][i]]
                nxt = [streams[st][i + 1] if i + 1 < nsteps else None
                       for st in (0, 1)]
                esbA = emit_pair_scores(p[0])
                la0 = emit_lepe(nxt[0]) if nxt[0] is not None else None
                emit_av_norm(p[0], 0, vna_cur[0], esbA[0])
                esbB = emit_pair_scores(p[1])
                emit_av_norm(p[0], 1, vna_cur[0], esbA[1])
                la1 = emit_lepe(nxt[1]) if nxt[1] is not None else None
                emit_av_norm(p[1], 0, vna_cur[1], esbB[0])
                if la0 is not None:
                    vna_cur[0] = emit_vna(nxt[0], emit_vlsb(nxt[0], *la0))
                emit_av_norm(p[1], 1, vna_cur[1], esbB[1])
                if la1 is not None:
                    vna_cur[1] = emit_vna(nxt[1], emit_vlsb(nxt[1], *la1))

        # ---------------- concatT assembly + proj ----------------
        with tc.tile_pool(name="concat", bufs=1) as concat_pool, \
             tc.tile_pool(name="vcm", bufs=3) as vcm_pool, \
             tc.tile_pool(name="pj_psum", bufs=4, space="PSUM") as pj_psum, \
             tc.tile_pool(name="pj", bufs=3) as pj:
            concatT = [concat_pool.tile([128, N], BF16, name=f"concatT{fc}",
                                        tag=f"concatT{fc}") for fc in range(4)]
            # h-half: TTc cols are already global token order -> 2 chunked
            # transposes per pair straight into concatT.
            for fc in range(2):
                for hc in range(2):
                    out_ap = bass.AP(
                        tensor=concatT[fc].tensor,
                        offset=concatT[fc].offset + hc * 2048,
                        ap=[concatT[fc].ap[0], [128, 16], [1, 128]])
                    nc.sync.dma_start_transpose(
                        out_ap, TTc[fc][:, hc * 2048:(hc + 1) * 2048])
            # v-half: per stripe: transpose to col-major staging, then DVE
            # scatter-copy into global token order.
            for fc in range(2, 4):
                for s in range(NS):
                    vcm = vcm_pool.tile([128, 4, 128], BF16, tag="vcm",
                                        name="vcm")
                    nc.sync.dma_start_transpose(
                        vcm, TTc[fc][:, s * 512:(s + 1) * 512])
                    out_ap = bass.AP(
                        tensor=concatT[fc].tensor,
                        offset=concatT[fc].offset + 8 * s,
                        ap=[concatT[fc].ap[0], [1, 8], [64, 64]])
                    nc.vector.tensor_copy(
                        out_ap, vcm.rearrange("p a b -> p (a b)"))

            # proj: 32 token chunks; bias via K=1 ones matmul
            osb = None
            for tt in range(32):
                ps = pj_psum.tile([128, C], F32, tag="pjps")
                for fcc in range(4):
                    nc.tensor.matmul(
                        ps, concatT[fcc][:, tt * 128:(tt + 1) * 128],
                        wprojT[fcc],
                        start=(fcc == 0), stop=False,
                        skip_group_check=True)
                nc.tensor.matmul(
                    ps, ones_row, bproj_sb,
                    start=False, stop=True, skip_group_check=True)
                if tt % 2 == 0:
                    osb = pj.tile([128, 2, C], F32, tag="pjout", name="pjout")
                    nc.scalar.activation(
                        osb[:, 0], ps, mybir.ActivationFunctionType.Copy)
                else:
                    nc.vector.tensor_copy(osb[:, 1], ps)
                    nc.sync.dma_start(
                        out=y_d.rearrange("(a p) c -> p a c", p=128)[
                            :, tt - 1:tt + 1, :],
                        in_=osb)


def _get_nc():
    if "nc" not in _CACHE:
        _CACHE["nc"] = _build_nc()
    return _CACHE["nc"]


def kernel(**inputs):
    x = np.asarray(inputs["x"], dtype=np.float32)
    names = {
        "wqkv": "Wqkv", "bqkv": "bqkv", "wproj": "Wproj", "bproj": "bproj",
        "lepe_h_w": "lepe_h_w", "lepe_h_b": "lepe_h_b",
        "lepe_v_w": "lepe_v_w", "lepe_v_b": "lepe_v_b",
    }
    shared = {k: np.ascontiguousarray(np.asarray(inputs[v], dtype=np.float32))
              for k, v in names.items()}
    nc = _get_nc()
    in_maps = []
    for b in range(B):
        m = dict(shared)
        m["x"] = np.ascontiguousarray(x[b])
        in_maps.append(m)
    res = bass_utils.run_bass_kernel_spmd(nc, in_maps, core_ids=list(range(B)))
    out = np.stack([res.results[b]["y"] for b in range(B)], axis=0)
    return out.astype(np.float32)


if __name__ == "__main__":
    rng = np.random.default_rng(0)
    ins = {
        "x": rng.standard_normal((B, N, C), dtype=np.float32),
        "Wqkv": rng.standard_normal((3 * C, C), dtype=np.float32) * C ** -0.5,
        "bqkv": np.zeros(3 * C, np.float32),
        "Wproj": rng.standard_normal((C, C), dtype=np.float32) * C ** -0.5,
        "bproj": np.zeros(C, np.float32),
        "lepe_h_w": rng.standard_normal((3, 3, 1, HD), dtype=np.float32) / 3,
        "lepe_h_b": np.zeros(HD, np.float32),
        "lepe_v_w": rng.standard_normal((3, 3, 1, HD), dtype=np.float32) / 3,
        "lepe_v_b": np.zeros(HD, np.float32),
        "H": np.int64(H), "W": np.int64(W),
    }
    out = kernel(**ins)
    print(out.shape, out.dtype)
